# revision 59
# baseline (speedup 1.0000x reference)
"""Trainium2 Bass kernel for batched cross-attention with attention-weight output.

Reference (per full problem):
    scores  = einsum('bqd,bkd->bqk', decoder_hidden, encoder_outputs)   # no scaling
    attn    = softmax(scores, axis=-1)
    context = einsum('bqk,bkd->bqd', attn, encoder_outputs)
    returns (context, attn)

Shapes: decoder_hidden [16, 1024, 1024] f32, encoder_outputs [16, 2048, 1024] f32.

Sharding: data-parallel over batch across 8 NeuronCores (2 batches per core).
Each core runs an identical single-core program on its slice; the host
concatenates the per-core outputs.

Per-core kernel (B_LOC=2, Q=1024, K=2048, D=1024):
  - enc natural layout [k(128p), 16, 1024] resident in SBUF (rhs of MM2),
    loaded with an f32->f32r rounding cast in-DMA (SWDGE)
  - encT [d(128p), 8, 2048] built with PE transposes (rhs of MM1), transpose
    outputs grouped 4-per-PSUM-bank so each PSUM->SBUF copy is 512 wide;
    copies alternate DVE/ACT so neither engine's backlog stalls the PE
  - per 128-row q tile: PE-transpose dh tile -> dhT; MM1 into 4 PSUM banks
    (dj-outer so consecutive matmuls share the stationary operand); softmax
    max-reduce (DVE) + exp with fused row-sum (ACT accum_out); the attnT
    transposes consume the UNNORMALIZED exp, the 1/sum lands for free in the
    context PSUM->SBUF copy (tensor_scalar_mul), and the attention-weight
    normalize + store runs off the critical path in the next iteration
  - MM2 ki-outer (stationary attnT[ki] reused across both 512-wide d chunks)
  - q loop is software-pipelined one tile deep so PE never waits on softmax;
    the next tile's dh load (on the ACT HWDGE ring, separate from the SP store
    ring) + transposes are prefetched so their single-engine copies drain
    during the previous tile's tail, and the first dh tile's transposes fill
    the PE while the batch's enc DMA streams in

Matmul input tiles use float32r (fp32 storage, single-pass PE matmul instead of
the 4-cycle/row fp32 path, ~11-bit effective mantissa); the producing copies /
cast-DMAs perform the f32r rounding the BIR verifier requires.
Measured: ~250-300 us/core on HW (differential in-program-REPEAT method;
CoreSim cost model predicts 326 us), rel err ~9.2e-04 vs the fp32 reference.
An MM-only microbenchmark (kernel_mmonly.py) paces the same 1024 matmuls at
~120-160 us on HW, so remaining time is transposes + inter-engine waits.
"""

import os

import numpy as np

import concourse.tile as tile
from concourse import bacc, mybir
from concourse.masks import make_identity

N_CORES = 8
B, Q, K, D = 16, 1024, 2048, 1024
B_LOC = B // N_CORES  # 2
P = 128
QT = Q // P        # 8 q tiles per batch
KT = K // P        # 16 k tiles
DT = D // P        # 8 d tiles
KC = K // 512      # 4 score chunks of 512
DC = D // 512      # 2 context chunks of 512

F32 = mybir.dt.float32
# float32r: fp32-storage matmul at full PE rate (vs 4 cycles/row for float32).
MM_DT = getattr(mybir.dt, os.environ.get("KERNEL_MM_DT", "float32r"))
# repeat whole computation in-program (timing aid; output unchanged)
REPEAT = int(os.environ.get("KERNEL_REPEAT", "1"))


def build_nc(repeat=None):
    repeat = REPEAT if repeat is None else repeat
    nc = bacc.Bacc("TRN2", target_bir_lowering=False)

    dh = nc.dram_tensor("decoder_hidden", [B_LOC, Q, D], F32, kind="ExternalInput")
    enc = nc.dram_tensor("encoder_outputs", [B_LOC, K, D], F32, kind="ExternalInput")
    ctx_out = nc.dram_tensor("context", [B_LOC, Q, D], F32, kind="ExternalOutput")
    attn_out = nc.dram_tensor("attn_w", [B_LOC, Q, K], F32, kind="ExternalOutput")

    cast_dma = MM_DT != F32

    with tile.TileContext(nc) as tc:
        with (
            tc.tile_pool(name="consts", bufs=1) as consts,
            tc.tile_pool(name="big", bufs=1) as big,
            tc.tile_pool(name="work", bufs=2) as work,
            tc.tile_pool(name="stats", bufs=4) as stats_pool,
            tc.tile_pool(name="psum_tr", bufs=2, space="PSUM") as psum_tr,
            tc.tile_pool(name="psum_s", bufs=KC, space="PSUM") as psum_s,
            tc.tile_pool(name="psum_c", bufs=1, space="PSUM") as psum_c,
        ):
            identity_f32 = consts.tile([P, P], F32)
            make_identity(nc, identity_f32)
            if cast_dma:
                identity = consts.tile([P, P], MM_DT)
                nc.vector.tensor_copy(identity[:], identity_f32[:])
            else:
                identity = identity_f32

            # state carried across loop iterations for 1-deep software pipeline
            prev = None  # (b, j, attn_sb, enc_nat, recip)

            def tail(prev_state):
                """Previous q tile: attnT transposes of the UNNORMALIZED exp,
                MM2, normalize-during-copy of context, then normalize + store
                the attention weights (off the critical path)."""
                pb, pj, p_attn, p_enc_nat, p_recip = prev_state
                attnT = work.tile([P, KT, P], MM_DT, tag="attnT")
                for g in range(KT // 4):
                    pt = psum_tr.tile([P, 4, P], MM_DT, tag="pt")
                    for i in range(4):
                        nc.tensor.transpose(
                            pt[:, i, :],
                            p_attn[:, (4 * g + i) * P:(4 * g + i + 1) * P],
                            identity,
                        )
                    if g % 2 == 0:
                        nc.vector.tensor_copy(attnT[:, 4 * g:4 * g + 4, :], pt[:])
                    else:
                        nc.scalar.copy(attnT[:, 4 * g:4 * g + 4, :], pt[:])
                ctx_sb = work.tile([P, D], F32, tag="ctx")
                pc = psum_c.tile([P, DC, 512], F32, tag="pc")
                for ki in range(KT):
                    for c2 in range(DC):
                        nc.tensor.matmul(
                            pc[:, c2, :],
                            attnT[:, ki, :],
                            p_enc_nat[:, ki, c2 * 512:(c2 + 1) * 512],
                            start=(ki == 0),
                            stop=(ki == KT - 1),
                        )
                nc.vector.tensor_scalar_mul(
                    ctx_sb[:], pc[:].rearrange("p a b -> p (a b)"), p_recip[:]
                )
                nc.sync.dma_start(ctx_out[pb, pj * P:(pj + 1) * P, :], ctx_sb[:])
                # normalize attention weights in place and store them
                nc.scalar.mul(p_attn[:], p_attn[:], p_recip[:])
                nc.sync.dma_start(
                    attn_out[pb, pj * P:(pj + 1) * P, :], p_attn[:].bitcast(F32)
                )

            for b_rep in range(B_LOC * repeat):
                b = b_rep % B_LOC
                # The two 64KB/partition enc regions ALTERNATE roles between
                # batch instances: this batch's natural-layout load lands in
                # the slot that held the previous batch's encT (dead after its
                # last MM1), so the 8 MB enc DMA overlaps the previous batch's
                # final attnT/MM2 tail instead of serializing behind it.
                s_nat, s_tr = b_rep % 2, 1 - (b_rep % 2)

                # ---- load enc (rounding cast in-DMA); WAR on the slot is
                # tracked by Tile, so this can be emitted before the flush
                enc_nat = big.tile(
                    [P, KT, D], MM_DT, tag=f"slot{s_nat}", name=f"nat{s_nat}"
                )
                enc_v = enc[b].rearrange("(ko p) d -> p ko d", p=P)
                for g in range(KT // 4):
                    if cast_dma:
                        nc.gpsimd.dma_start(
                            enc_nat[:, 4 * g:4 * g + 4, :], enc_v[:, 4 * g:4 * g + 4, :]
                        )
                    else:
                        nc.sync.dma_start(
                            enc_nat[:, 4 * g:4 * g + 4, :], enc_v[:, 4 * g:4 * g + 4, :]
                        )

                # flush the pipeline (previous batch's last tile reads its
                # enc_nat, which this batch's encT will overwrite)
                if prev is not None:
                    tail(prev)
                    prev = None

                # ---- build encT with PE transposes
                encT = big.tile(
                    [P, DT, K], MM_DT, tag=f"slot{s_tr}", name=f"tr{s_tr}"
                )
                def dh_prefetch(jj):
                    """Load dh tile jj and transpose to dhT [d, q]. Emitted one
                    iteration ahead so the PSUM->SBUF copies (single engine ->
                    single wait for MM1) drain during the previous tile's tail.
                    (dh feeds only transposes, so it stays f32 on HWDGE; the
                    copy performs the f32r rounding.)"""
                    dh_tile = work.tile([P, D], F32, tag="dh", name=f"dh{jj}")
                    # ACT HWDGE ring: don't queue behind attn/ctx stores on SP
                    nc.scalar.dma_start(dh_tile[:], dh[b, jj * P:(jj + 1) * P, :])
                    dhT = work.tile([P, DT, P], MM_DT, tag="dhT", name=f"dhT{jj}")
                    for g in range(DT // 4):
                        pt = psum_tr.tile([P, 4, P], F32, tag="pt", name=f"ptd{g}")
                        for i in range(4):
                            nc.tensor.transpose(
                                pt[:, i, :],
                                dh_tile[:, (4 * g + i) * P:(4 * g + i + 1) * P],
                                identity_f32,
                            )
                        nc.vector.tensor_copy(dhT[:, 4 * g:4 * g + 4, :], pt[:])
                    return dhT

                dhT_next = dh_prefetch(0)

                for g in range(KT // 4):  # k-group of 4 tiles = 512 k values
                    for dj in range(DT):
                        pt = psum_tr.tile([P, 4, P], MM_DT, tag="pt")
                        for i in range(4):
                            nc.tensor.transpose(
                                pt[:, i, :],
                                enc_nat[:, 4 * g + i, dj * P:(dj + 1) * P],
                                identity,
                            )
                        if dj % 2 == 0:
                            nc.vector.tensor_copy(
                                encT[:, dj, g * 512:(g + 1) * 512],
                                pt[:].rearrange("p a b -> p (a b)"),
                            )
                        else:
                            nc.scalar.copy(
                                encT[:, dj, g * 512:(g + 1) * 512],
                                pt[:].rearrange("p a b -> p (a b)"),
                            )

                for j in range(QT):
                    dhT = dhT_next

                    # ---- MM1: scores[q, k] accumulated over d into 4 PSUM banks
                    # dj-outer so 4 consecutive matmuls share the stationary
                    # operand dhT[:, dj, :]
                    scores = [
                        psum_s.tile([P, 512], F32, tag="ps", name=f"ps{c}")
                        for c in range(KC)
                    ]
                    for dj in range(DT):
                        for c in range(KC):
                            nc.tensor.matmul(
                                scores[c][:],
                                dhT[:, dj, :],
                                encT[:, dj, c * 512:(c + 1) * 512],
                                start=(dj == 0),
                                stop=(dj == DT - 1),
                            )

                    # ---- prefetch next tile's dh transposes: their copies
                    # drain while the PE runs the previous tile's tail
                    if j + 1 < QT:
                        dhT_next = dh_prefetch(j + 1)

                    # ---- previous q tile's attnT + MM2 run on PE while this
                    # tile's softmax runs on DVE/ACT
                    if prev is not None:
                        tail(prev)

                    # ---- softmax over k (free dim); attn_sb holds the
                    # UNNORMALIZED exp — normalization happens in tail()
                    cmax = stats_pool.tile([P, KC], F32, tag="cmax")
                    for c in range(KC):
                        nc.vector.reduce_max(
                            cmax[:, c:c + 1], scores[c][:], axis=mybir.AxisListType.X
                        )
                    negmax = stats_pool.tile([P, 1], F32, tag="negmax")
                    nc.vector.reduce_max(
                        negmax[:], cmax[:], axis=mybir.AxisListType.X, negate=True
                    )
                    attn_sb = work.tile([P, K], MM_DT, tag="attn")
                    sums = stats_pool.tile([P, KC], F32, tag="sums")
                    for c in range(KC):
                        nc.scalar.activation(
                            attn_sb[:, c * 512:(c + 1) * 512],
                            scores[c][:],
                            mybir.ActivationFunctionType.Exp,
                            bias=negmax[:],
                            accum_out=sums[:, c:c + 1],
                        )
                    ssum = stats_pool.tile([P, 1], F32, tag="ssum")
                    nc.vector.reduce_sum(ssum[:], sums[:], axis=mybir.AxisListType.X)
                    recip = stats_pool.tile([P, 1], F32, tag="recip")
                    nc.vector.reciprocal(recip[:], ssum[:])

                    prev = (b, j, attn_sb, enc_nat, recip)

            tail(prev)

    nc.compile()
    return nc


_RUNNER_CACHE = None


def _build_runner():
    """Sharded PJRT runner for the bass program: one jitted shard_map call
    over 8 cores, with the NEFF's output buffers created on-device (zeros)
    instead of being shipped from the host."""
    import jax
    from jax.sharding import Mesh, NamedSharding, PartitionSpec
    from jax.experimental.shard_map import shard_map
    import jax.numpy as jnp

    from concourse import bass2jax, mybir as _mybir
    from concourse.bass2jax import _bass_exec_p, install_neuronx_cc_hook

    nc = build_nc()
    install_neuronx_cc_hook()

    partition_name = nc.partition_id_tensor.name if nc.partition_id_tensor else None
    in_names, out_names, out_avals = [], [], []
    for alloc in nc.m.functions[0].allocations:
        if not isinstance(alloc, _mybir.MemoryLocationSet):
            continue
        name = alloc.memorylocations[0].name
        if alloc.kind == "ExternalInput":
            if name != partition_name:
                in_names.append(name)
        elif alloc.kind == "ExternalOutput":
            out_names.append(name)
            out_avals.append(
                jax.core.ShapedArray(
                    tuple(alloc.tensor_shape), _mybir.dt.np(alloc.dtype)
                )
            )
    all_in_names = list(in_names) + list(out_names)
    if partition_name is not None:
        all_in_names.append(partition_name)

    def _body(*args):
        operands = list(args)
        if partition_name is not None:
            operands.append(bass2jax.partition_id_tensor())
        return tuple(
            _bass_exec_p.bind(
                *operands,
                out_avals=tuple(out_avals),
                in_names=tuple(all_in_names),
                out_names=tuple(out_names),
                lowering_input_output_aliases=(),
                sim_require_finite=True,
                sim_require_nnan=True,
                nc=nc,
            )
        )

    devices = jax.devices()[:N_CORES]
    mesh = Mesh(np.asarray(devices), ("core",))
    sh = NamedSharding(mesh, PartitionSpec("core"))
    n_in = len(in_names)
    n_out = len(out_names)
    fn = jax.jit(
        shard_map(
            _body,
            mesh=mesh,
            in_specs=(PartitionSpec("core"),) * (n_in + n_out),
            out_specs=(PartitionSpec("core"),) * n_out,
            check_rep=False,
        ),
        keep_unused=True,
    )

    # zero output buffers, created on-device (no 192 MB host->device ship)
    def _mk_zeros():
        return tuple(
            jnp.zeros((N_CORES * av.shape[0], *av.shape[1:]), av.dtype)
            for av in out_avals
        )

    zeros_dev = jax.jit(_mk_zeros, out_shardings=(sh,) * n_out)()
    return fn, in_names, out_names, zeros_dev


def kernel(decoder_hidden: np.ndarray, encoder_outputs: np.ndarray):
    global _RUNNER_CACHE
    if _RUNNER_CACHE is None:
        _RUNNER_CACHE = _build_runner()
    fn, in_names, out_names, zeros_dev = _RUNNER_CACHE

    arrs = {
        "decoder_hidden": np.ascontiguousarray(
            np.asarray(decoder_hidden, dtype=np.float32)
        ),
        "encoder_outputs": np.ascontiguousarray(
            np.asarray(encoder_outputs, dtype=np.float32)
        ),
    }
    outs = fn(*[arrs[n] for n in in_names], *zeros_dev)
    by_name = {n: np.asarray(o) for n, o in zip(out_names, outs)}
    return by_name["context"], by_name["attn_w"]


# revision 62
# speedup vs baseline: 1.4810x; 1.4810x over previous
"""Trainium2 Bass kernel for batched cross-attention with attention-weight output.

Reference (per full problem):
    scores  = einsum('bqd,bkd->bqk', decoder_hidden, encoder_outputs)   # no scaling
    attn    = softmax(scores, axis=-1)
    context = einsum('bqk,bkd->bqd', attn, encoder_outputs)
    returns (context, attn)

Shapes: decoder_hidden [16, 1024, 1024] f32, encoder_outputs [16, 2048, 1024] f32.

Sharding: data-parallel over batch across 8 NeuronCores (2 batches per core).
Each core runs an identical single-core program on its slice; the host
concatenates the per-core outputs.

Per-core kernel (B_LOC=2, Q=1024, K=2048, D=1024):
  - enc natural layout [k(128p), 16, 1024] resident in SBUF (rhs of MM2),
    loaded with an f32->f32r rounding cast in-DMA (SWDGE)
  - encT [d(128p), 8, 2048] built with PE transposes (rhs of MM1), transpose
    outputs grouped 4-per-PSUM-bank so each PSUM->SBUF copy is 512 wide;
    copies alternate DVE/ACT so neither engine's backlog stalls the PE
  - per 128-row q tile: PE-transpose dh tile -> dhT; MM1 into 4 PSUM banks
    (dj-outer so consecutive matmuls share the stationary operand); softmax
    max-reduce (DVE) + exp with fused row-sum (ACT accum_out); the attnT
    transposes consume the UNNORMALIZED exp, the 1/sum lands for free in the
    context PSUM->SBUF copy (tensor_scalar_mul), and the attention-weight
    normalize + store runs off the critical path in the next iteration
  - MM2 ki-outer (stationary attnT[ki] reused across both 512-wide d chunks)
  - q loop is software-pipelined one tile deep so PE never waits on softmax;
    the next tile's dh load (on the ACT HWDGE ring, separate from the SP store
    ring) + transposes are prefetched so their single-engine copies drain
    during the previous tile's tail, and the first dh tile's transposes fill
    the PE while the batch's enc DMA streams in

Matmul input tiles use float32r (fp32 storage, single-pass PE matmul instead of
the 4-cycle/row fp32 path, ~11-bit effective mantissa); the producing copies /
cast-DMAs perform the f32r rounding the BIR verifier requires.
The two 64KB/partition enc SBUF regions alternate natural/transposed roles
between batch instances, so each batch's 8 MB enc DMA starts as soon as the
previous batch's last MM1 retires (encT dead) and overlaps its final tail.
Measured: ~200-250 us/core on HW (differential in-program-REPEAT method;
CoreSim cost model predicts 316 us), rel err ~9.2e-04 vs the fp32 reference.
Microbenchmarks: the same 1024 matmuls alone pace at ~120-160 us
(kernel_mmonly.py); the 640 transposes + copies at ~71 us (kernel_tronly.py).
"""

import os

import numpy as np

import concourse.tile as tile
from concourse import bacc, mybir
from concourse.masks import make_identity

N_CORES = 8
B, Q, K, D = 16, 1024, 2048, 1024
B_LOC = B // N_CORES  # 2
P = 128
QT = Q // P        # 8 q tiles per batch
KT = K // P        # 16 k tiles
DT = D // P        # 8 d tiles
KC = K // 512      # 4 score chunks of 512
DC = D // 512      # 2 context chunks of 512

F32 = mybir.dt.float32
# float32r: fp32-storage matmul at full PE rate (vs 4 cycles/row for float32).
MM_DT = getattr(mybir.dt, os.environ.get("KERNEL_MM_DT", "float32r"))
# repeat whole computation in-program (timing aid; output unchanged)
REPEAT = int(os.environ.get("KERNEL_REPEAT", "1"))


def build_nc(repeat=None):
    repeat = REPEAT if repeat is None else repeat
    nc = bacc.Bacc("TRN2", target_bir_lowering=False)

    dh = nc.dram_tensor("decoder_hidden", [B_LOC, Q, D], F32, kind="ExternalInput")
    enc = nc.dram_tensor("encoder_outputs", [B_LOC, K, D], F32, kind="ExternalInput")
    ctx_out = nc.dram_tensor("context", [B_LOC, Q, D], F32, kind="ExternalOutput")
    attn_out = nc.dram_tensor("attn_w", [B_LOC, Q, K], F32, kind="ExternalOutput")

    cast_dma = MM_DT != F32

    with tile.TileContext(nc) as tc:
        with (
            tc.tile_pool(name="consts", bufs=1) as consts,
            tc.tile_pool(name="big", bufs=1) as big,
            tc.tile_pool(name="work", bufs=2) as work,
            tc.tile_pool(name="stats", bufs=4) as stats_pool,
            tc.tile_pool(name="psum_tr", bufs=2, space="PSUM") as psum_tr,
            tc.tile_pool(name="psum_s", bufs=KC, space="PSUM") as psum_s,
            tc.tile_pool(name="psum_c", bufs=1, space="PSUM") as psum_c,
        ):
            identity_f32 = consts.tile([P, P], F32)
            make_identity(nc, identity_f32)
            if cast_dma:
                identity = consts.tile([P, P], MM_DT)
                nc.vector.tensor_copy(identity[:], identity_f32[:])
            else:
                identity = identity_f32

            # state carried across loop iterations for 1-deep software pipeline
            prev = None  # (b, j, attn_sb, enc_nat, recip)

            def tail(prev_state):
                """Previous q tile: attnT transposes of the UNNORMALIZED exp,
                MM2, normalize-during-copy of context, then normalize + store
                the attention weights (off the critical path)."""
                pb, pj, p_attn, p_enc_nat, p_recip = prev_state
                attnT = work.tile([P, KT, P], MM_DT, tag="attnT")
                for g in range(KT // 4):
                    pt = psum_tr.tile([P, 4, P], MM_DT, tag="pt")
                    for i in range(4):
                        nc.tensor.transpose(
                            pt[:, i, :],
                            p_attn[:, (4 * g + i) * P:(4 * g + i + 1) * P],
                            identity,
                        )
                    if g % 2 == 0:
                        nc.vector.tensor_copy(attnT[:, 4 * g:4 * g + 4, :], pt[:])
                    else:
                        nc.scalar.copy(attnT[:, 4 * g:4 * g + 4, :], pt[:])
                ctx_sb = work.tile([P, D], F32, tag="ctx")
                pc = psum_c.tile([P, DC, 512], F32, tag="pc")
                for ki in range(KT):
                    for c2 in range(DC):
                        nc.tensor.matmul(
                            pc[:, c2, :],
                            attnT[:, ki, :],
                            p_enc_nat[:, ki, c2 * 512:(c2 + 1) * 512],
                            start=(ki == 0),
                            stop=(ki == KT - 1),
                        )
                nc.vector.tensor_scalar_mul(
                    ctx_sb[:], pc[:].rearrange("p a b -> p (a b)"), p_recip[:]
                )
                nc.sync.dma_start(ctx_out[pb, pj * P:(pj + 1) * P, :], ctx_sb[:])
                # normalize attention weights in place and store them
                nc.scalar.mul(p_attn[:], p_attn[:], p_recip[:])
                nc.sync.dma_start(
                    attn_out[pb, pj * P:(pj + 1) * P, :], p_attn[:].bitcast(F32)
                )

            for b_rep in range(B_LOC * repeat):
                b = b_rep % B_LOC
                # The two 64KB/partition enc regions ALTERNATE roles between
                # batch instances: this batch's natural-layout load lands in
                # the slot that held the previous batch's encT (dead after its
                # last MM1), so the 8 MB enc DMA overlaps the previous batch's
                # final attnT/MM2 tail instead of serializing behind it.
                s_nat, s_tr = b_rep % 2, 1 - (b_rep % 2)

                # ---- load enc (rounding cast in-DMA); WAR on the slot is
                # tracked by Tile, so this can be emitted before the flush
                enc_nat = big.tile(
                    [P, KT, D], MM_DT, tag=f"slot{s_nat}", name=f"nat{s_nat}"
                )
                enc_v = enc[b].rearrange("(ko p) d -> p ko d", p=P)
                for g in range(KT // 4):
                    if cast_dma:
                        nc.gpsimd.dma_start(
                            enc_nat[:, 4 * g:4 * g + 4, :], enc_v[:, 4 * g:4 * g + 4, :]
                        )
                    else:
                        nc.sync.dma_start(
                            enc_nat[:, 4 * g:4 * g + 4, :], enc_v[:, 4 * g:4 * g + 4, :]
                        )

                # flush the pipeline (previous batch's last tile reads its
                # enc_nat, which this batch's encT will overwrite)
                if prev is not None:
                    tail(prev)
                    prev = None

                # ---- build encT with PE transposes
                encT = big.tile(
                    [P, DT, K], MM_DT, tag=f"slot{s_tr}", name=f"tr{s_tr}"
                )
                def dh_prefetch(jj):
                    """Load dh tile jj and transpose to dhT [d, q]. Emitted one
                    iteration ahead so the PSUM->SBUF copies (single engine ->
                    single wait for MM1) drain during the previous tile's tail.
                    (dh feeds only transposes, so it stays f32 on HWDGE; the
                    copy performs the f32r rounding.)"""
                    dh_tile = work.tile([P, D], F32, tag="dh", name=f"dh{jj}")
                    # ACT HWDGE ring: don't queue behind attn/ctx stores on SP
                    nc.scalar.dma_start(dh_tile[:], dh[b, jj * P:(jj + 1) * P, :])
                    dhT = work.tile([P, DT, P], MM_DT, tag="dhT", name=f"dhT{jj}")
                    for g in range(DT // 4):
                        pt = psum_tr.tile([P, 4, P], F32, tag="pt", name=f"ptd{g}")
                        for i in range(4):
                            nc.tensor.transpose(
                                pt[:, i, :],
                                dh_tile[:, (4 * g + i) * P:(4 * g + i + 1) * P],
                                identity_f32,
                            )
                        nc.vector.tensor_copy(dhT[:, 4 * g:4 * g + 4, :], pt[:])
                    return dhT

                dhT_next = dh_prefetch(0)

                # enc transposes interleaved with the FIRST q tile's MM1: each
                # k-chunk of scores needs only its own encT k-chunk, so MM1
                # chunk g runs right after transpose group g — the PE does
                # useful matmul work while later enc chunks are still landing
                scores0 = []
                for g in range(KT // 4):  # k-group of 4 tiles = 512 k values
                    for dj in range(DT):
                        pt = psum_tr.tile([P, 4, P], MM_DT, tag="pt")
                        for i in range(4):
                            nc.tensor.transpose(
                                pt[:, i, :],
                                enc_nat[:, 4 * g + i, dj * P:(dj + 1) * P],
                                identity,
                            )
                        if dj % 2 == 0:
                            nc.vector.tensor_copy(
                                encT[:, dj, g * 512:(g + 1) * 512],
                                pt[:].rearrange("p a b -> p (a b)"),
                            )
                        else:
                            nc.scalar.copy(
                                encT[:, dj, g * 512:(g + 1) * 512],
                                pt[:].rearrange("p a b -> p (a b)"),
                            )
                    ps0 = psum_s.tile([P, 512], F32, tag="ps", name=f"ps0{g}")
                    for dj in range(DT):
                        nc.tensor.matmul(
                            ps0[:],
                            dhT_next[:, dj, :],
                            encT[:, dj, g * 512:(g + 1) * 512],
                            start=(dj == 0),
                            stop=(dj == DT - 1),
                        )
                    scores0.append(ps0)

                for j in range(QT):
                    dhT = dhT_next

                    # ---- MM1: scores[q, k] accumulated over d into 4 PSUM banks
                    # dj-outer so 4 consecutive matmuls share the stationary
                    # operand dhT[:, dj, :] (tile 0's MM1 already ran,
                    # interleaved with the enc transposes above)
                    if j == 0:
                        scores = scores0
                    else:
                        scores = [
                            psum_s.tile([P, 512], F32, tag="ps", name=f"ps{c}")
                            for c in range(KC)
                        ]
                        for dj in range(DT):
                            for c in range(KC):
                                nc.tensor.matmul(
                                    scores[c][:],
                                    dhT[:, dj, :],
                                    encT[:, dj, c * 512:(c + 1) * 512],
                                    start=(dj == 0),
                                    stop=(dj == DT - 1),
                                )

                    # ---- prefetch next tile's dh transposes: their copies
                    # drain while the PE runs the previous tile's tail
                    if j + 1 < QT:
                        dhT_next = dh_prefetch(j + 1)

                    # ---- previous q tile's attnT + MM2 run on PE while this
                    # tile's softmax runs on DVE/ACT
                    if prev is not None:
                        tail(prev)

                    # ---- softmax over k (free dim); attn_sb holds the
                    # UNNORMALIZED exp — normalization happens in tail()
                    cmax = stats_pool.tile([P, KC], F32, tag="cmax")
                    for c in range(KC):
                        nc.vector.reduce_max(
                            cmax[:, c:c + 1], scores[c][:], axis=mybir.AxisListType.X
                        )
                    negmax = stats_pool.tile([P, 1], F32, tag="negmax")
                    nc.vector.reduce_max(
                        negmax[:], cmax[:], axis=mybir.AxisListType.X, negate=True
                    )
                    attn_sb = work.tile([P, K], MM_DT, tag="attn")
                    sums = stats_pool.tile([P, KC], F32, tag="sums")
                    for c in range(KC):
                        nc.scalar.activation(
                            attn_sb[:, c * 512:(c + 1) * 512],
                            scores[c][:],
                            mybir.ActivationFunctionType.Exp,
                            bias=negmax[:],
                            accum_out=sums[:, c:c + 1],
                        )
                    ssum = stats_pool.tile([P, 1], F32, tag="ssum")
                    nc.vector.reduce_sum(ssum[:], sums[:], axis=mybir.AxisListType.X)
                    recip = stats_pool.tile([P, 1], F32, tag="recip")
                    nc.vector.reciprocal(recip[:], ssum[:])

                    prev = (b, j, attn_sb, enc_nat, recip)

            tail(prev)

    nc.compile()
    return nc


_RUNNER_CACHE = None


def _build_runner():
    """Sharded PJRT runner for the bass program: one jitted shard_map call
    over 8 cores, with the NEFF's output buffers created on-device (zeros)
    instead of being shipped from the host."""
    import jax
    from jax.sharding import Mesh, NamedSharding, PartitionSpec
    from jax.experimental.shard_map import shard_map
    import jax.numpy as jnp

    from concourse import bass2jax, mybir as _mybir
    from concourse.bass2jax import _bass_exec_p, install_neuronx_cc_hook

    nc = build_nc()
    install_neuronx_cc_hook()

    partition_name = nc.partition_id_tensor.name if nc.partition_id_tensor else None
    in_names, out_names, out_avals = [], [], []
    for alloc in nc.m.functions[0].allocations:
        if not isinstance(alloc, _mybir.MemoryLocationSet):
            continue
        name = alloc.memorylocations[0].name
        if alloc.kind == "ExternalInput":
            if name != partition_name:
                in_names.append(name)
        elif alloc.kind == "ExternalOutput":
            out_names.append(name)
            out_avals.append(
                jax.core.ShapedArray(
                    tuple(alloc.tensor_shape), _mybir.dt.np(alloc.dtype)
                )
            )
    all_in_names = list(in_names) + list(out_names)
    if partition_name is not None:
        all_in_names.append(partition_name)

    def _body(*args):
        operands = list(args)
        if partition_name is not None:
            operands.append(bass2jax.partition_id_tensor())
        return tuple(
            _bass_exec_p.bind(
                *operands,
                out_avals=tuple(out_avals),
                in_names=tuple(all_in_names),
                out_names=tuple(out_names),
                lowering_input_output_aliases=(),
                sim_require_finite=True,
                sim_require_nnan=True,
                nc=nc,
            )
        )

    devices = jax.devices()[:N_CORES]
    mesh = Mesh(np.asarray(devices), ("core",))
    sh = NamedSharding(mesh, PartitionSpec("core"))
    n_in = len(in_names)
    n_out = len(out_names)
    fn = jax.jit(
        shard_map(
            _body,
            mesh=mesh,
            in_specs=(PartitionSpec("core"),) * (n_in + n_out),
            out_specs=(PartitionSpec("core"),) * n_out,
            check_rep=False,
        ),
        keep_unused=True,
    )

    # zero output buffers, created on-device (no 192 MB host->device ship)
    def _mk_zeros():
        return tuple(
            jnp.zeros((N_CORES * av.shape[0], *av.shape[1:]), av.dtype)
            for av in out_avals
        )

    zeros_dev = jax.jit(_mk_zeros, out_shardings=(sh,) * n_out)()
    return fn, in_names, out_names, zeros_dev


def kernel(decoder_hidden: np.ndarray, encoder_outputs: np.ndarray):
    global _RUNNER_CACHE
    if _RUNNER_CACHE is None:
        _RUNNER_CACHE = _build_runner()
    fn, in_names, out_names, zeros_dev = _RUNNER_CACHE

    arrs = {
        "decoder_hidden": np.ascontiguousarray(
            np.asarray(decoder_hidden, dtype=np.float32)
        ),
        "encoder_outputs": np.ascontiguousarray(
            np.asarray(encoder_outputs, dtype=np.float32)
        ),
    }
    outs = fn(*[arrs[n] for n in in_names], *zeros_dev)
    by_name = {n: np.asarray(o) for n, o in zip(out_names, outs)}
    return by_name["context"], by_name["attn_w"]
